# revision 3
# baseline (speedup 1.0000x reference)
"""Self-contained Trainium2 Bass kernel for single-head self-attention.

Problem: inputs [8, 2048, 512] f32, Wq/Wk/Wv [512, 512], bq/bk/bv [512].
Returns (output [8, 2048, 512], weights [8, 2048, 2048]) matching

    q = x @ Wq + bq ; k = x @ Wk + bk ; v = x @ Wv + bv
    scores = q @ k.T / sqrt(E)
    weights = softmax(scores, axis=-1)
    output = weights @ v

Sharding: data-parallel over batch — one batch element per NeuronCore,
8 cores. Each core runs an identical Bass program on its own slice.

Per-core dataflow (S=2048 sequence, E=512 embed, P=128 partitions):
  - load x tiles [128, 512], PE-transpose (fp32 mode) into xT [e, seq]
  - kT = Wk.T @ x.T, qT = Wq.T @ x.T (psum->sbuf copy adds bias),
    v = x @ Wv (natural layout, bias added via broadcast row)
  - per 128-row query chunk: scores = qT.T @ kT into PSUM, exp via
    ScalarE activation (scale folded; no max subtraction — scores are
    O(6) so fp32 exp cannot overflow) with accumulated row sums,
    normalized weights DMA'd out, exp tiles PE-transposed to feed
    out = (exp @ v) * recip(rowsum).
Contraction matmuls run in float32r (full-rate fp32 streaming mode);
the BIR verifier requires every fp32r matmul input to be written by a
compute instruction with float32r output dtype, so the existing
psum->SBUF copies double as the rounding step and the DMA-loaded
projection weights get one explicit rounding copy each.
"""

import numpy as np

B, S, E, P = 8, 2048, 512, 128
NI = S // P      # 16 query/key chunks of 128
NK = E // P      # 4 embed chunks of 128
NS = S // 512    # 4 column splits of the score row
SCALE = 1.0 / float(np.sqrt(E))

_CACHE = {}


def _build_nc():
    import concourse.bass as bass
    import concourse.tile as tile
    from concourse import bacc, mybir
    from concourse.masks import make_identity

    f32 = mybir.dt.float32
    f32r = mybir.dt.float32r

    nc = bacc.Bacc("TRN2", target_bir_lowering=False)

    x_h = nc.dram_tensor("x", (S, E), f32, kind="ExternalInput")
    wq_h = nc.dram_tensor("wq", (E, E), f32, kind="ExternalInput")
    bq_h = nc.dram_tensor("bq", (E,), f32, kind="ExternalInput")
    wk_h = nc.dram_tensor("wk", (E, E), f32, kind="ExternalInput")
    bk_h = nc.dram_tensor("bk", (E,), f32, kind="ExternalInput")
    wv_h = nc.dram_tensor("wv", (E, E), f32, kind="ExternalInput")
    bv_h = nc.dram_tensor("bv", (E,), f32, kind="ExternalInput")
    out_h = nc.dram_tensor("out", (S, E), f32, kind="ExternalOutput")
    wts_h = nc.dram_tensor("wts", (S, S), f32, kind="ExternalOutput")

    xr = x_h[:].rearrange("(n p) e -> n p e", p=P)       # [16, 128, 512]
    wqr = wq_h[:].rearrange("(c p) e -> c p e", p=P)     # [4, 128, 512]
    wkr = wk_h[:].rearrange("(c p) e -> c p e", p=P)
    wvr = wv_h[:].rearrange("(c p) e -> c p e", p=P)
    outr = out_h[:].rearrange("(n p) e -> n p e", p=P)
    wtsr = wts_h[:].rearrange("(n p) j -> n p j", p=P)

    AX = mybir.AxisListType
    EXP = mybir.ActivationFunctionType.Exp
    ADD = mybir.AluOpType.add

    with tile.TileContext(nc) as tc:
        with tc.tile_pool(name="persist", bufs=1) as persist:
            qT = persist.tile([P, NK * S], f32r)  # [:, c*S + i] = q[i, c*128+p]
            kT = persist.tile([P, NK * S], f32r)
            v = persist.tile([P, NI * E], f32r)   # [:, j*E + e] = v[j*128+p, e]
            ident = persist.tile([P, P], f32)
            make_identity(nc, ident[:])
            bqt = persist.tile([P, NK], f32)      # [:, c] = bq[c*128+p]
            bkt = persist.tile([P, NK], f32)
            bv_row = persist.tile([1, E], f32)
            ones = persist.tile([1, P], f32)
            bvb = persist.tile([P, E], f32)       # bv broadcast to 128 partitions

            with tc.tile_pool(name="ldx", bufs=1) as ldx, \
                 tc.tile_pool(name="stg", bufs=3) as stg, \
                 tc.tile_pool(name="psA", bufs=1, space=bass.MemorySpace.PSUM) as psA, \
                 tc.tile_pool(name="psB", bufs=4, space=bass.MemorySpace.PSUM) as psB:
                xT = ldx.tile([P, NK * S], f32r)  # [:, c*S + i] = x[i, c*128+p]
                wq_sb = ldx.tile([P, NK * E], f32r)  # [:, c*E+e] = Wq[c*128+p, e]
                wk_sb = ldx.tile([P, NK * E], f32r)
                wv_sb = ldx.tile([P, NK * E], f32r)

                # ---- load x and transpose into xT (fp32-mode transpose,
                # the psum->sbuf copy rounds into f32r) ----
                for i in range(NI):
                    xs = stg.tile([P, E], f32, tag="xs")
                    nc.sync.dma_start(xs[:], xr[i])
                    tp = psA.tile([P, E], f32, tag="tp")
                    for c in range(NK):
                        nc.tensor.transpose(
                            tp[:, c * P : (c + 1) * P],
                            xs[:, c * P : (c + 1) * P],
                            ident[:],
                        )
                    dst = xT[:].rearrange("p (c s) -> p c s", c=NK)[
                        :, :, i * P : (i + 1) * P
                    ]
                    src = tp[:].rearrange("p (c q) -> p c q", c=NK)
                    if i % 2 == 0:
                        nc.vector.tensor_copy(dst, src)
                    else:
                        nc.scalar.copy(dst, src)

                # ---- projection weights: DMA then round to f32r ----
                with tc.tile_pool(name="ldraw", bufs=1) as ldraw:
                    for w_sb, wr in ((wq_sb, wqr), (wk_sb, wkr), (wv_sb, wvr)):
                        raw = ldraw.tile([P, NK * E], f32, tag="raw")
                        for c in range(NK):
                            nc.sync.dma_start(raw[:, c * E : (c + 1) * E], wr[c])
                        nc.scalar.copy(w_sb[:], raw[:])

                # ---- biases ----
                for c in range(NK):
                    nc.sync.dma_start(bqt[:, c : c + 1], bq_h[c * P : (c + 1) * P])
                    nc.sync.dma_start(bkt[:, c : c + 1], bk_h[c * P : (c + 1) * P])
                nc.sync.dma_start(bv_row[:], bv_h[:])
                nc.gpsimd.memset(ones[:], 1.0)
                bc_ps = psA.tile([P, E], f32, tag="bc")
                nc.tensor.matmul(bc_ps[:], ones[:], bv_row[:], start=True, stop=True)
                nc.vector.tensor_copy(bvb[:], bc_ps[:])

                # ---- kT, qT, v projections ----
                for c in range(NK):      # output embed chunk
                    for n in range(NS):  # 512-wide seq window
                        kp = psB.tile([P, 512], f32, tag="pb")
                        for kk in range(NK):
                            nc.tensor.matmul(
                                kp[:],
                                wk_sb[:, kk * E + c * P : kk * E + (c + 1) * P],
                                xT[:, kk * S + n * 512 : kk * S + (n + 1) * 512],
                                start=(kk == 0),
                                stop=(kk == NK - 1),
                            )
                        nc.scalar.add(
                            kT[:, c * S + n * 512 : c * S + (n + 1) * 512],
                            kp[:],
                            bkt[:, c : c + 1],
                        )
                for c in range(NK):
                    for n in range(NS):
                        qp = psB.tile([P, 512], f32, tag="pb")
                        for kk in range(NK):
                            nc.tensor.matmul(
                                qp[:],
                                wq_sb[:, kk * E + c * P : kk * E + (c + 1) * P],
                                xT[:, kk * S + n * 512 : kk * S + (n + 1) * 512],
                                start=(kk == 0),
                                stop=(kk == NK - 1),
                            )
                        nc.scalar.add(
                            qT[:, c * S + n * 512 : c * S + (n + 1) * 512],
                            qp[:],
                            bqt[:, c : c + 1],
                        )
                for j in range(NI):
                    vp = psB.tile([P, E], f32, tag="pb")
                    for kk in range(NK):
                        nc.tensor.matmul(
                            vp[:],
                            xT[:, kk * S + j * P : kk * S + (j + 1) * P],
                            wv_sb[:, kk * E : (kk + 1) * E],
                            start=(kk == 0),
                            stop=(kk == NK - 1),
                        )
                    nc.vector.tensor_tensor(
                        out=v[:, j * E : (j + 1) * E], in0=vp[:], in1=bvb[:], op=ADD
                    )

            # ---- attention main loop: one 128-query chunk at a time ----
            with tc.tile_pool(name="work", bufs=2) as work, \
                 tc.tile_pool(name="wtp", bufs=4) as wtp, \
                 tc.tile_pool(name="psC", bufs=2, space=bass.MemorySpace.PSUM) as psC, \
                 tc.tile_pool(name="psT", bufs=4, space=bass.MemorySpace.PSUM) as psT:
                for i in range(NI):
                    wtile = work.tile([P, S], f32, tag="wtile")  # exp(scores)
                    sums = work.tile([P, NS], f32, tag="sums")
                    for n in range(NS):
                        sc = psC.tile([P, 512], f32, tag="sc")
                        for c in range(NK):
                            nc.tensor.matmul(
                                sc[:],
                                qT[:, c * S + i * P : c * S + (i + 1) * P],
                                kT[:, c * S + n * 512 : c * S + (n + 1) * 512],
                                start=(c == 0),
                                stop=(c == NK - 1),
                            )
                        nc.scalar.activation(
                            wtile[:, n * 512 : (n + 1) * 512],
                            sc[:],
                            EXP,
                            scale=SCALE,
                            accum_out=sums[:, n : n + 1],
                        )
                    rcp = work.tile([P, 1], f32, tag="rcp")
                    rowsum = work.tile([P, 1], f32, tag="rowsum")
                    nc.vector.reduce_sum(rowsum[:], sums[:], axis=AX.X)
                    nc.vector.reciprocal(rcp[:], rowsum[:])
                    wnorm = work.tile([P, S], f32, tag="wnorm")
                    nc.vector.tensor_scalar_mul(wnorm[:], wtile[:], rcp[:])
                    nc.sync.dma_start(wtsr[i], wnorm[:])

                    # transpose exp tiles (fp32 mode), round into f32r on the
                    # psum->sbuf copy, then out += expT.T @ v
                    wTs = []
                    for g in range(4):
                        tr = psT.tile([P, 512], f32, tag="tr")
                        for t in range(4):
                            j = g * 4 + t
                            nc.tensor.transpose(
                                tr[:, t * P : (t + 1) * P],
                                wtile[:, j * P : (j + 1) * P],
                                ident[:],
                            )
                        wT = wtp.tile([P, 512], f32r, tag="wT")
                        nc.vector.tensor_copy(wT[:], tr[:])
                        wTs.append(wT)
                    out_ps = psC.tile([P, E], f32, tag="ops")
                    for g in range(4):
                        for t in range(4):
                            j = g * 4 + t
                            nc.tensor.matmul(
                                out_ps[:],
                                wTs[g][:, t * P : (t + 1) * P],
                                v[:, j * E : (j + 1) * E],
                                start=(j == 0),
                                stop=(j == NI - 1),
                            )
                    outsb = work.tile([P, E], f32, tag="outsb")
                    nc.scalar.mul(outsb[:], out_ps[:], rcp[:])
                    nc.sync.dma_start(outr[i], outsb[:])

    nc.compile()
    return nc


def _get_nc():
    if "nc" not in _CACHE:
        _CACHE["nc"] = _build_nc()
    return _CACHE["nc"]


def _run_spmd(inputs, **kwargs):
    from concourse.bass_utils import run_bass_kernel_spmd

    nc = _get_nc()
    x = np.asarray(inputs["inputs"], dtype=np.float32)
    in_maps = []
    for b in range(B):
        in_maps.append(
            {
                "x": np.ascontiguousarray(x[b]),
                "wq": np.asarray(inputs["Wq"], dtype=np.float32),
                "bq": np.asarray(inputs["bq"], dtype=np.float32),
                "wk": np.asarray(inputs["Wk"], dtype=np.float32),
                "bk": np.asarray(inputs["bk"], dtype=np.float32),
                "wv": np.asarray(inputs["Wv"], dtype=np.float32),
                "bv": np.asarray(inputs["bv"], dtype=np.float32),
            }
        )
    res = run_bass_kernel_spmd(nc, in_maps, core_ids=list(range(B)), **kwargs)
    output = np.stack([res.results[c]["out"] for c in range(B)])
    weights = np.stack([res.results[c]["wts"] for c in range(B)])
    return output, weights, res


def kernel(**inputs):
    output, weights, _ = _run_spmd(inputs)
    return output, weights


# revision 16
# speedup vs baseline: 51.5431x; 51.5431x over previous
"""Self-contained Trainium2 Bass kernel for single-head self-attention.

Problem: inputs [8, 2048, 512] f32, Wq/Wk/Wv [512, 512], bq/bk/bv [512].
Returns (output [8, 2048, 512], weights [8, 2048, 2048]) matching

    q = x @ Wq + bq ; k = x @ Wk + bk ; v = x @ Wv + bv
    scores = q @ k.T / sqrt(E)
    weights = softmax(scores, axis=-1)
    output = weights @ v

Sharding: data-parallel over batch — one batch element per NeuronCore,
8 cores. Each core runs an identical Bass program on its own slice.

Per-core dataflow (S=2048 sequence, E=512 embed, P=128 partitions):
  - load x tiles [128, 512], PE-transpose (fp32 mode) into xT [e, seq]
  - kT = Wk.T @ x.T, qT = Wq.T @ x.T (psum->sbuf copy adds bias),
    v = x @ Wv (natural layout, bias added via broadcast row)
  - per 128-row query chunk: scores = qT.T @ kT into PSUM, exp via
    ScalarE activation (scale folded; no max subtraction — scores are
    O(6) so fp32 exp cannot overflow) with accumulated row sums,
    normalized weights DMA'd out, exp tiles PE-transposed to feed
    out = (exp @ v) * recip(rowsum).
Contraction matmuls run in float32r (full-rate fp32 streaming mode);
the BIR verifier requires every fp32r matmul input to be written by a
compute instruction with float32r output dtype, so the existing
psum->SBUF copies double as the rounding step and the DMA-loaded
projection weights get one explicit rounding copy each.
"""

import numpy as np

B, S, E, P = 8, 2048, 512, 128
NI = S // P      # 16 query/key chunks of 128
NK = E // P      # 4 embed chunks of 128
NS = S // 512    # 4 column splits of the score row
SCALE = 1.0 / float(np.sqrt(E))

_CACHE = {}


def _build_nc():
    import concourse.bass as bass
    import concourse.tile as tile
    from concourse import bacc, mybir
    from concourse.masks import make_identity

    f32 = mybir.dt.float32
    f32r = mybir.dt.float32r

    nc = bacc.Bacc("TRN2", target_bir_lowering=False)

    x_h = nc.dram_tensor("x", (S, E), f32, kind="ExternalInput")
    wq_h = nc.dram_tensor("wq", (E, E), f32, kind="ExternalInput")
    bq_h = nc.dram_tensor("bq", (E,), f32, kind="ExternalInput")
    wk_h = nc.dram_tensor("wk", (E, E), f32, kind="ExternalInput")
    bk_h = nc.dram_tensor("bk", (E,), f32, kind="ExternalInput")
    wv_h = nc.dram_tensor("wv", (E, E), f32, kind="ExternalInput")
    bv_h = nc.dram_tensor("bv", (E,), f32, kind="ExternalInput")
    out_h = nc.dram_tensor("out", (S, E), f32, kind="ExternalOutput")
    wts_h = nc.dram_tensor("wts", (S, S), f32, kind="ExternalOutput")

    xr = x_h[:].rearrange("(n p) e -> n p e", p=P)       # [16, 128, 512]
    wqr = wq_h[:].rearrange("(c p) e -> c p e", p=P)     # [4, 128, 512]
    wkr = wk_h[:].rearrange("(c p) e -> c p e", p=P)
    wvr = wv_h[:].rearrange("(c p) e -> c p e", p=P)
    outr = out_h[:].rearrange("(n p) e -> n p e", p=P)
    wtsr = wts_h[:].rearrange("(n p) j -> n p j", p=P)

    AX = mybir.AxisListType
    EXP = mybir.ActivationFunctionType.Exp
    ADD = mybir.AluOpType.add

    with tile.TileContext(nc) as tc:
        with tc.tile_pool(name="persist", bufs=1) as persist:
            qT = persist.tile([P, NK * S], f32r)  # [:, c*S + i] = q[i, c*128+p]
            kT = persist.tile([P, NK * S], f32r)
            v = persist.tile([P, NI * E], f32r)   # [:, j*E + e] = v[j*128+p, e]
            ident = persist.tile([P, P], f32)
            make_identity(nc, ident[:])
            bqt = persist.tile([P, NK], f32)      # [:, c] = bq[c*128+p]
            bkt = persist.tile([P, NK], f32)
            bv_row = persist.tile([1, E], f32)
            ones = persist.tile([1, P], f32)
            bvb = persist.tile([P, E], f32)       # bv broadcast to 128 partitions

            with tc.tile_pool(name="ldx", bufs=1) as ldx, \
                 tc.tile_pool(name="stg", bufs=8) as stg, \
                 tc.tile_pool(name="ldraw", bufs=1) as ldraw, \
                 tc.tile_pool(name="psA", bufs=2, space=bass.MemorySpace.PSUM) as psA, \
                 tc.tile_pool(name="psB", bufs=4, space=bass.MemorySpace.PSUM) as psB:
                xT = ldx.tile([P, NK * S], f32r)  # [:, c*S + i] = x[i, c*128+p]
                wq_sb = ldx.tile([P, NK * E], f32r)  # [:, c*E+e] = Wq[c*128+p, e]
                wk_sb = ldx.tile([P, NK * E], f32r)
                wv_sb = ldx.tile([P, NK * E], f32r)

                # x window-0 tiles first (PE's first work), then W, biases,
                # then the remaining x windows. Column-split DMAs let each
                # transpose start as soon as its 64KB lands.
                xs_tiles = []
                xrc = x_h[:].rearrange(
                    "(n p) (c q) -> n c p q", p=P, q=P
                )  # [16, 4, 128, 128]
                for i in range(NI):
                    xs = stg.tile([P, E], f32, tag="xs")
                    xs_tiles.append(xs)

                def load_x(i):
                    for c in range(NK):
                        nc.sync.dma_start(
                            xs_tiles[i][:, c * P : (c + 1) * P], xrc[i, c]
                        )

                for i in range(4):
                    load_x(i)
                raws = []
                for wr in (wkr, wqr, wvr):
                    raw = ldraw.tile([P, NK * E], f32, tag="raw" + str(len(raws)))
                    for c in range(NK):
                        nc.sync.dma_start(raw[:, c * E : (c + 1) * E], wr[c])
                    raws.append(raw)
                for c in range(NK):
                    nc.sync.dma_start(bqt[:, c : c + 1], bq_h[c * P : (c + 1) * P])
                    nc.sync.dma_start(bkt[:, c : c + 1], bk_h[c * P : (c + 1) * P])
                nc.sync.dma_start(bv_row[:], bv_h[:])
                nc.gpsimd.memset(ones[:], 1.0)
                for i in range(4, NI):
                    load_x(i)
                for c in range(NK):
                    sl = slice(c * E, (c + 1) * E)
                    nc.scalar.copy(wk_sb[:, sl], raws[0][:, sl])
                    nc.scalar.copy(wq_sb[:, sl], raws[1][:, sl])
                    nc.vector.tensor_copy(wv_sb[:, sl], raws[2][:, sl])
                bc_ps = psA.tile([P, E], f32, tag="bc")
                nc.tensor.matmul(bc_ps[:], ones[:], bv_row[:], start=True, stop=True)
                nc.vector.tensor_copy(bvb[:], bc_ps[:])

                # ---- per 512-wide seq window: transpose x, then project.
                # Software-pipelined emission: window n+1's transposes and
                # xT copies are emitted before window n's projections, so
                # the in-order ACT/DVE streams finish the copies while the
                # PE runs the previous window's matmuls. ----
                def emit_transposes(n):
                    for i in range(4 * n, 4 * n + 4):
                        xs = xs_tiles[i]
                        tp = psA.tile([P, E], f32, tag="tp")
                        for c in range(NK):
                            nc.tensor.transpose(
                                tp[:, c * P : (c + 1) * P],
                                xs[:, c * P : (c + 1) * P],
                                ident[:],
                            )
                        dst = xT[:].rearrange("p (c s) -> p c s", c=NK)[
                            :, :, i * P : (i + 1) * P
                        ]
                        src = tp[:].rearrange("p (c q) -> p c q", c=NK)
                        if i % 2 == 0:
                            nc.vector.tensor_copy(dst, src)
                        else:
                            nc.scalar.copy(dst, src)

                emit_transposes(0)
                for n in range(NS):
                    if n + 1 < NS:
                        emit_transposes(n + 1)
                    for c in range(NK):      # kT for this window
                        kp = psB.tile([P, 512], f32, tag="pb")
                        for kk in range(NK):
                            nc.tensor.matmul(
                                kp[:],
                                wk_sb[:, kk * E + c * P : kk * E + (c + 1) * P],
                                xT[:, kk * S + n * 512 : kk * S + (n + 1) * 512],
                                start=(kk == 0),
                                stop=(kk == NK - 1),
                            )
                        nc.scalar.add(
                            kT[:, c * S + n * 512 : c * S + (n + 1) * 512],
                            kp[:],
                            bkt[:, c : c + 1],
                        )
                    for c in range(NK):      # qT for this window
                        qp = psB.tile([P, 512], f32, tag="pb")
                        for kk in range(NK):
                            nc.tensor.matmul(
                                qp[:],
                                wq_sb[:, kk * E + c * P : kk * E + (c + 1) * P],
                                xT[:, kk * S + n * 512 : kk * S + (n + 1) * 512],
                                start=(kk == 0),
                                stop=(kk == NK - 1),
                            )
                        nc.scalar.add(
                            qT[:, c * S + n * 512 : c * S + (n + 1) * 512],
                            qp[:],
                            bqt[:, c : c + 1],
                        )
                    for j in range(4 * n, 4 * n + 4):  # v for this window
                        vp = psB.tile([P, E], f32, tag="pb")
                        for kk in range(NK):
                            nc.tensor.matmul(
                                vp[:],
                                xT[:, kk * S + j * P : kk * S + (j + 1) * P],
                                wv_sb[:, kk * E : (kk + 1) * E],
                                start=(kk == 0),
                                stop=(kk == NK - 1),
                            )
                        nc.vector.tensor_tensor(
                            out=v[:, j * E : (j + 1) * E], in0=vp[:], in1=bvb[:], op=ADD
                        )

            # ---- attention main loop: one 128-query chunk at a time ----
            with tc.tile_pool(name="work", bufs=2) as work, \
                 tc.tile_pool(name="wtp", bufs=4) as wtp, \
                 tc.tile_pool(name="psC", bufs=2, space=bass.MemorySpace.PSUM) as psC, \
                 tc.tile_pool(name="psT", bufs=4, space=bass.MemorySpace.PSUM) as psT:
                # Two-stage software pipeline over query chunks: the PE runs
                # chunk i+1's score matmuls while ScalarE finishes chunk i's
                # exp, so the weight transposes never wait on the activation.
                state = {}

                def emit_scores(i):
                    wtile = work.tile([P, S], f32, tag="wtile")  # exp(scores)
                    sums = work.tile([P, NS], f32, tag="sums")
                    for n in range(NS):
                        sc = psC.tile([P, 512], f32, tag="sc")
                        for c in range(NK):
                            nc.tensor.matmul(
                                sc[:],
                                qT[:, c * S + i * P : c * S + (i + 1) * P],
                                kT[:, c * S + n * 512 : c * S + (n + 1) * 512],
                                start=(c == 0),
                                stop=(c == NK - 1),
                            )
                        nc.scalar.activation(
                            wtile[:, n * 512 : (n + 1) * 512],
                            sc[:],
                            EXP,
                            scale=SCALE,
                            accum_out=sums[:, n : n + 1],
                        )
                    state[i] = (wtile, sums)

                def emit_attend(i):
                    wtile, sums = state.pop(i)
                    rcp = work.tile([P, 1], f32, tag="rcp")
                    rowsum = work.tile([P, 1], f32, tag="rowsum")
                    nc.vector.reduce_sum(rowsum[:], sums[:], axis=AX.X)
                    nc.vector.reciprocal(rcp[:], rowsum[:])
                    wnorm = work.tile([P, S], f32, tag="wnorm")
                    for h in range(2):
                        sl = slice(h * (S // 2), (h + 1) * (S // 2))
                        nc.vector.tensor_scalar_mul(
                            wnorm[:, sl], wtile[:, sl], rcp[:]
                        )
                        nc.sync.dma_start(wtsr[i][:, sl], wnorm[:, sl])

                    # transpose exp tiles (fp32 mode), round into f32r on the
                    # psum->sbuf copy, then out += expT.T @ v
                    wTs = []
                    for g in range(4):
                        tr = psT.tile([P, 512], f32, tag="tr")
                        for t in range(4):
                            j = g * 4 + t
                            nc.tensor.transpose(
                                tr[:, t * P : (t + 1) * P],
                                wtile[:, j * P : (j + 1) * P],
                                ident[:],
                            )
                        wT = wtp.tile([P, 512], f32r, tag="wT")
                        nc.vector.tensor_copy(wT[:], tr[:])
                        wTs.append(wT)
                    out_ps = psC.tile([P, E], f32, tag="ops")
                    for g in range(4):
                        for t in range(4):
                            j = g * 4 + t
                            nc.tensor.matmul(
                                out_ps[:],
                                wTs[g][:, t * P : (t + 1) * P],
                                v[:, j * E : (j + 1) * E],
                                start=(j == 0),
                                stop=(j == NI - 1),
                            )
                    outsb = work.tile([P, E], f32, tag="outsb")
                    nc.scalar.mul(outsb[:], out_ps[:], rcp[:])
                    nc.sync.dma_start(outr[i], outsb[:])

                emit_scores(0)
                for i in range(NI):
                    if i + 1 < NI:
                        emit_scores(i + 1)
                    emit_attend(i)

    nc.compile()
    return nc


def _get_nc():
    if "nc" not in _CACHE:
        _CACHE["nc"] = _build_nc()
    return _CACHE["nc"]


def _run_spmd(inputs, **kwargs):
    from concourse.bass_utils import run_bass_kernel_spmd

    nc = _get_nc()
    x = np.asarray(inputs["inputs"], dtype=np.float32)
    in_maps = []
    for b in range(B):
        in_maps.append(
            {
                "x": np.ascontiguousarray(x[b]),
                "wq": np.asarray(inputs["Wq"], dtype=np.float32),
                "bq": np.asarray(inputs["bq"], dtype=np.float32),
                "wk": np.asarray(inputs["Wk"], dtype=np.float32),
                "bk": np.asarray(inputs["bk"], dtype=np.float32),
                "wv": np.asarray(inputs["Wv"], dtype=np.float32),
                "bv": np.asarray(inputs["bv"], dtype=np.float32),
            }
        )
    res = run_bass_kernel_spmd(nc, in_maps, core_ids=list(range(B)), **kwargs)
    output = np.stack([res.results[c]["out"] for c in range(B)])
    weights = np.stack([res.results[c]["wts"] for c in range(B)])
    return output, weights, res


def kernel(**inputs):
    output, weights, _ = _run_spmd(inputs)
    return output, weights
